# revision 1
# baseline (speedup 1.0000x reference)
"""Cross-attention Trainium2 kernel (nn_CrossAttention).

Reference computation (per batch b):
    q = Wq @ x1 + bq            [32, N]     (N = 64*64 = 4096)
    k = Wk @ x2 + bk            [32, N]
    v = Wv @ x2 + bv            [256, N]
    attn = softmax(q^T k, axis over keys m)     [N, N]
    out[c, n] = sum_m v[c, m] attn[n, m]        [256, N]

Sharding: 8 cores = 4 batches x 2 query-halves (2048 queries per core, all
4096 keys).  Each core runs the same NEFF on its own input slice; softmax
rows are complete within a core so no cross-core communication is needed.

Per-core kernel layout choices:
  * Inputs x1/x2 and weights are fp16 on the host side: projections run at
    full PE rate (1 cyc/row) with half the HBM traffic of fp32, and fp16's
    11-bit mantissa keeps q/k logits accurate (bf16 inputs push the final
    rel-err past 1.5e-2; fp16 keeps it ~3e-3).
  * S^T tiles [keys m on partitions, queries n on free dim] so the second
    matmul (attn @ V) consumes exp(S^T) directly from SBUF with m as the
    contraction dim -- no transposes anywhere.
  * Q and K are produced replicated 4x across partition groups (Wq/Wk
    stacked 4x on the host) so the D=32-contraction QK^T matmuls can be
    row-packed 4-per-PE-array via tile_position.
  * The S^T PSUM is split into two [128, 1024] halves on a 2-deep pool:
    exp of half h (step i) overlaps the S^T matmuls of step i+1 instead of
    serializing behind them (single-buffer psum was the baseline's pacer).
  * exp(S^T) is written in bf16 by the ACT engine only (exact exp); all
    PSUM evacuation / normalization work runs on DVE + GpSimd so ACT never
    stalls the softmax pipeline.
  * Row-sum partials land on partitions {0,32,64,96} via col-packed M=1
    ones-matmuls; a DMA gather + K=4 ones-matmul combines and broadcasts
    them, then a fast approximate reciprocal normalizes.
  * Softmax skips the max-subtraction: logits are ~N(0, 32), |s| < ~48
    for this problem size, exp() stays comfortably inside fp32/bf16 range.
  * bv is folded in at the end: out += bv (softmax rows sum to 1).
"""

import sys

for _p in (
    "/root/.axon_site",
    "/root/.axon_site/_ro/trn_rl_repo",
    "/root/.axon_site/_ro/pypackages",
):
    if _p not in sys.path:
        sys.path.append(_p)

import numpy as np

import concourse.bass as bass
from concourse import bacc
import concourse.tile as tile
from concourse import mybir
from concourse import bass_utils

B = 4
C = 256          # value/input channels
D = 32           # q/k channels
N = 4096         # keys per batch (64*64)
NQ = 2048        # queries per core (half a batch)
NT = 512         # query tile (free dim of S^T / output matmuls)
NNT = NQ // NT   # 4 query tiles
NSC = 8          # key super-chunks of 512 (4 x 128) keys
F32 = mybir.dt.float32
F32R = mybir.dt.float32r
F16 = mybir.dt.float16
BF16 = mybir.dt.bfloat16
AFT = mybir.ActivationFunctionType


def attn_tile_kernel(tc, out, x1, x2, wall, biases, ones_c, ones_f):
    nc = tc.nc

    with (
        tc.tile_pool(name="consts", bufs=1) as consts,
        tc.tile_pool(name="bigbuf", bufs=1) as bigbuf,
        # 4 pt bufs: two halves are allocated per step BEFORE the previous
        # step's AV consumers are emitted; with <4 bufs a new exp would
        # reuse a buffer whose reader isn't emitted yet (untracked race).
        tc.tile_pool(name="ptbuf", bufs=4) as ptbuf,
        tc.tile_pool(name="finbuf", bufs=2) as finbuf,
    ):
        # ---- constants / weights -------------------------------------
        ones_rs = consts.tile([128, 32], BF16, name="ones_rs")
        nc.scalar.dma_start(out=ones_rs, in_=ones_c)
        ones_bc = consts.tile([128, 128], F32R, name="ones_bc")
        nc.scalar.dma_start(out=ones_bc, in_=ones_f)

        # biases packed as one [128, 4] f32: cols = bq4 | bk4 | bv0 | bv1
        bias_sb = consts.tile([128, 4], F32, name="bias_sb")
        nc.scalar.dma_start(out=bias_sb, in_=biases)
        bq4_sb = bias_sb[:, 0:1]
        bk4_sb = bias_sb[:, 1:2]
        bv_sb = [bias_sb[:, 2:3], bias_sb[:, 3:4]]

        # All weights ride in one contiguous [128, 1024] fp16 tensor (one
        # 2KB-per-partition DMA instead of ~800 tiny descriptors), already
        # host-interleaved to the channel-pair layout: plane ch holds
        # channels {2p+ch}, cols = [wq 128 | wk 128 | wv 256] per plane.
        wall_sb = consts.tile([128, 1024], F16, name="wall_sb")
        nc.scalar.dma_start(out=wall_sb, in_=wall)
        wq4t_sb = [wall_sb[:, ch * 512 : ch * 512 + 128] for ch in range(2)]
        wk4t_sb = [wall_sb[:, ch * 512 + 128 : ch * 512 + 256] for ch in range(2)]
        wvt_sb = [wall_sb[:, ch * 512 + 256 : ch * 512 + 512] for ch in range(2)]

        # ---- feature maps -------------------------------------------
        # Host passes x1/x2 reshaped [128, 2*cols]: partition p holds the
        # channel pair (2p, 2p+1) back-to-back, so each DMA line is one
        # fully contiguous 8-16KB read (vs 128 small strided descriptors).
        # x1 gets the sync queue to itself (Q-proj is the critical path).
        # Per-queue DMA tops out at ~75GB/s, so parallelize: x1 split over
        # two queues (arrives in ~7us for Q-proj), x2 streamed in 1024-col
        # blocks alternating gpsimd/scalar so K/V prep unblocks per block.
        x1_sb = bigbuf.tile([128, 2 * NQ], F16, name="x1_sb")
        x2_sb = bigbuf.tile([128, 2 * N], F16, name="x2_sb")
        nc.sync.dma_start(out=x1_sb[0:64, :], in_=x1[0:64, :])
        nc.scalar.dma_start(out=x1_sb[64:128, :], in_=x1[64:128, :])
        for blk in range(4):
            # gpsimd DMA is software-DGE (slow descriptor gen): keep to the
            # two hardware DGE queues, alternating blocks
            eng = nc.sync if blk % 2 == 0 else nc.scalar
            for ch in range(2):
                cols = slice(ch * N + blk * 1024, ch * N + (blk + 1) * 1024)
                eng.dma_start(out=x2_sb[:, cols], in_=x2[:, cols])

        def x1p(ch, cols):
            return x1_sb[:, ch * NQ + cols.start : ch * NQ + cols.stop]

        def x2p(ch, cols):
            return x2_sb[:, ch * N + cols.start : ch * N + cols.stop]

        q4_sb = bigbuf.tile([128, NQ], F32R, name="q4_sb")
        k4_sb = bigbuf.tile([128, N], F32R, name="k4_sb")
        vt_sb = bigbuf.tile([128, C * N // 128], BF16, name="vt_sb")  # [128, 8192]

        # ---- prep: projections ---------------------------------------
        # Interleaved per 1024-column x2 block so PE work becomes available
        # as each DMA block lands: Q4 first (x1), then per block K4 + V^T.
        with tc.tile_pool(name="prep_psum", bufs=2, space="PSUM") as pp:
            # Q4 [128, 2048] = (Wq stacked 4x) @ x1, then +bq
            psum_q = pp.tile([128, NQ], F32, name="psum_q", tag="prep")
            for t4 in range(NNT):
                cols = slice(t4 * NT, (t4 + 1) * NT)
                for ch in range(2):
                    nc.tensor.matmul(
                        psum_q[:, cols],
                        lhsT=wq4t_sb[ch],
                        rhs=x1p(ch, cols),
                        start=(ch == 0),
                        stop=(ch == 1),
                    )
            nc.vector.tensor_scalar_add(q4_sb, psum_q, bq4_sb)

            for blk in range(4):
                bcols = slice(blk * 1024, (blk + 1) * 1024)
                # K4 for this block
                psum_k = pp.tile([128, 1024], F32, name=f"psum_k{blk}", tag="prep")
                for t2 in range(2):
                    cols = slice(t2 * NT, (t2 + 1) * NT)
                    src_c = slice(blk * 1024 + t2 * NT, blk * 1024 + (t2 + 1) * NT)
                    for ch in range(2):
                        nc.tensor.matmul(
                            psum_k[:, cols],
                            lhsT=wk4t_sb[ch],
                            rhs=x2p(ch, src_c),
                            start=(ch == 0),
                            stop=(ch == 1),
                        )
                nc.vector.tensor_scalar_add(k4_sb[:, bcols], psum_k, bk4_sb)
                # V^T (bf16) for this block's 8 m-chunks
                psum_v = pp.tile([128, 2048], F32, name=f"psum_v{blk}", tag="prep")
                for m8 in range(8):
                    mc = 8 * blk + m8
                    for ch in range(2):
                        nc.tensor.matmul(
                            psum_v[:, m8 * 256 : (m8 + 1) * 256],
                            lhsT=x2p(ch, slice(mc * 128, (mc + 1) * 128)),
                            rhs=wvt_sb[ch],
                            start=(ch == 0),
                            stop=(ch == 1),
                        )
                for h in range(2):
                    cols = slice(h * 1024, (h + 1) * 1024)
                    dst = vt_sb[:, blk * 2048 + h * 1024 : blk * 2048 + (h + 1) * 1024]
                    if h == 0:
                        nc.scalar.copy(dst, psum_v[:, cols])
                    else:
                        nc.vector.tensor_copy(dst, psum_v[:, cols])

        # ---- main attention loop -------------------------------------
        # Flat software pipeline over (nt, sc) steps, each split into two
        # halves h of 2 key-chunks.  The S^T psum is a 2-deep pool of
        # [128, 1024] halves, so exp (ACT) of half (i, h) overlaps the S^T
        # matmuls of the next half/step on the PE instead of serializing.
        # AV/rowsum matmuls of step i are emitted after step i+1's S^T, so
        # the PE always has work while ACT computes exp.
        with (
            tc.tile_pool(name="s_psum", bufs=2, space="PSUM") as sp,
            tc.tile_pool(name="o_psum", bufs=1, space="PSUM") as op,
            tc.tile_pool(name="b_psum", bufs=1, space="PSUM") as bp,
        ):
            state = {}

            def _emit_st_half(nt, sc, h):
                # S^T half: 2 row-packed matmuls (chunks 4*sc+2h+{0,1}) at
                # row-groups {2h, 2h+1}; exp -> bf16 pt on ACT.
                qcols = slice(nt * NT, (nt + 1) * NT)
                psum_s = sp.tile([128, 2 * NT], F32, name=f"ps_{nt}_{sc}_{h}", tag="s")
                for j in range(2):
                    mc = 4 * sc + 2 * h + j
                    rowg = slice(32 * (2 * h + j), 32 * (2 * h + j + 1))
                    nc.tensor.matmul(
                        psum_s[:, j * NT : (j + 1) * NT],
                        lhsT=k4_sb[rowg, mc * 128 : (mc + 1) * 128],
                        rhs=q4_sb[rowg, qcols],
                        start=True,
                        stop=True,
                        tile_position=(32 * (2 * h + j), 0),
                    )
                pt = ptbuf.tile([128, 2 * NT], BF16, name=f"pt_{nt}_{sc}_{h}", tag="pt")
                nc.scalar.activation(out=pt, in_=psum_s, func=AFT.Exp)
                return pt

            def _emit_rs(nt, sc, pt_a, pt_b):
                # col-packed rowsums: 4 concurrent M=1 tiles emitted
                # back-to-back (adjacency is required for the PE to
                # co-execute tile-disjoint matmuls); partials land on
                # partitions {0, 32, 64, 96}.
                if sc == 0:
                    state[nt] = (
                        op.tile([128, NT], F32, name=f"po0_{nt}", tag="o0"),
                        op.tile([128, NT], F32, name=f"po1_{nt}", tag="o1"),
                        op.tile([128, NT], F32, name=f"prs_{nt}", tag="rs"),
                    )
                psum_rs = state[nt][2]
                for g in range(4):
                    h, j = divmod(g, 2)
                    nc.tensor.matmul(
                        psum_rs[32 * g : 32 * (g + 1), :],
                        lhsT=ones_rs,
                        rhs=(pt_a, pt_b)[h][:, j * NT : (j + 1) * NT],
                        start=(sc == 0),
                        stop=(sc == NSC - 1),
                        tile_position=(0, 32 * g),
                        skip_group_check=True,
                    )

            def _emit_av_half(nt, sc, h, pt):
                first, last = (sc == 0 and h == 0), (sc == NSC - 1 and h == 1)
                psum_o0, psum_o1, _ = state[nt]
                for j in range(2):
                    mc = 4 * sc + 2 * h + j
                    pcols = slice(j * NT, (j + 1) * NT)
                    for cc in range(2):
                        nc.tensor.matmul(
                            (psum_o0, psum_o1)[cc],
                            lhsT=vt_sb[
                                :, mc * 256 + cc * 128 : mc * 256 + (cc + 1) * 128
                            ],
                            rhs=pt[:, pcols],
                            start=(first and j == 0),
                            stop=(last and j == 1),
                        )

            def _emit_fin(nt):
                # evacuate PSUM fast (frees banks for the next tile), then
                # normalize on SBUF; everything off the ACT engine so exp
                # never stalls.
                psum_o0, psum_o1, psum_rs = state.pop(nt)
                qcols = slice(nt * NT, (nt + 1) * NT)
                rs_sb = finbuf.tile([128, NT], F32R, name=f"rs_sb_{nt}", tag="rs_sb")
                nc.vector.tensor_copy(rs_sb, psum_rs)
                # GpSimd has no PSUM port: psum evacuation stays on ACT/DVE
                # (one ACT copy per 8-step tile fits in ACT's slack).
                raw0 = finbuf.tile([128, NT], F32, name=f"raw0_{nt}", tag="raw0")
                nc.scalar.copy(raw0, psum_o0)
                raw1 = finbuf.tile([128, NT], F32, name=f"raw1_{nt}", tag="raw1")
                nc.vector.tensor_copy(raw1, psum_o1)
                # gather the 4 partial rows onto adjacent partitions, then a
                # K=4 ones-matmul combines + broadcasts to all 128 partitions
                rs4p = finbuf.tile([4, NT], F32R, name=f"rs4p_{nt}", tag="rs4p")
                nc.sync.dma_start(out=rs4p, in_=rs_sb[0:97:32, :])
                psum_b = bp.tile([128, NT], F32, name=f"pb_{nt}", tag="b")
                nc.tensor.matmul(
                    psum_b, lhsT=ones_bc[0:4, :], rhs=rs4p, start=True, stop=True
                )
                rbc = finbuf.tile([128, NT], F32, name=f"rbc_{nt}", tag="rbc")
                nc.vector.reciprocal_approx_fast(out=rbc, in_=psum_b)
                # keep element-wise work on DVE: GpSimd's software tensor ops
                # measure ~15x slower than DVE on hardware
                for cc, raw in ((0, raw0), (1, raw1)):
                    t_sb = finbuf.tile([128, NT], F32, name=f"t_{nt}_{cc}", tag=f"t{cc}")
                    nc.vector.tensor_mul(t_sb, raw, rbc)
                    o_sb = finbuf.tile([128, NT], F32, name=f"o_{nt}_{cc}", tag=f"o{cc}")
                    nc.vector.tensor_scalar_add(o_sb, t_sb, bv_sb[cc])
                    # split the 512KB output across two queues: the final
                    # tile's write-out is the kernel's exposed tail
                    (nc.sync, nc.scalar)[cc].dma_start(
                        out=out[cc * 128 : (cc + 1) * 128, qcols], in_=o_sb
                    )

            steps = [(nt, sc) for nt in range(NNT) for sc in range(NSC)]
            prev = None
            for nt, sc in steps:
                pt_a = _emit_st_half(nt, sc, 0)
                pt_b = _emit_st_half(nt, sc, 1)
                if prev is not None:
                    pnt, psc, ppa, ppb = prev
                    _emit_rs(pnt, psc, ppa, ppb)
                    _emit_av_half(pnt, psc, 0, ppa)
                    _emit_av_half(pnt, psc, 1, ppb)
                    if psc == NSC - 1:
                        _emit_fin(pnt)
                prev = (nt, sc, pt_a, pt_b)
            pnt, psc, ppa, ppb = prev
            _emit_rs(pnt, psc, ppa, ppb)
            _emit_av_half(pnt, psc, 0, ppa)
            _emit_av_half(pnt, psc, 1, ppb)
            _emit_fin(pnt)


def build_nc():
    nc = bacc.Bacc("TRN2", target_bir_lowering=False, debug=False)
    x1 = nc.dram_tensor("x1", [128, 2 * NQ], F16, kind="ExternalInput").ap()
    x2 = nc.dram_tensor("x2", [128, 2 * N], F16, kind="ExternalInput").ap()
    wall = nc.dram_tensor("wall", [128, 1024], F16, kind="ExternalInput").ap()
    biases = nc.dram_tensor("biases", [128, 4], F32, kind="ExternalInput").ap()
    ones_cd = nc.dram_tensor("ones_c", [128, 32], BF16, kind="ExternalInput").ap()
    ones_fd = nc.dram_tensor("ones_f", [128, 128], F32R, kind="ExternalInput").ap()
    out = nc.dram_tensor("out", [C, NQ], F32, kind="ExternalOutput").ap()
    with tile.TileContext(nc) as tc:
        attn_tile_kernel(tc, out, x1, x2, wall, biases, ones_cd, ones_fd)
    nc.compile()
    return nc


def make_in_maps(f1, f2, Wq, bq, Wk, bk, Wv, bv):
    f1 = np.asarray(f1, dtype=np.float32)
    f2 = np.asarray(f2, dtype=np.float32)
    Wq = np.asarray(Wq, dtype=np.float32)
    Wk = np.asarray(Wk, dtype=np.float32)
    Wv = np.asarray(Wv, dtype=np.float32)
    bq = np.asarray(bq, dtype=np.float32)
    bk = np.asarray(bk, dtype=np.float32)
    bv = np.asarray(bv, dtype=np.float32)

    x1 = f1.reshape(B, C, N).astype(np.float16)
    x2 = f2.reshape(B, C, N).astype(np.float16)
    # channel-pair packing: SBUF partition p holds channels (2p, 2p+1), so
    # weight plane ch = rows (2p+ch) of W^T.  All weights merge into one
    # [128, 1024] fp16 tensor: per plane [wq4 128 | wk4 128 | wv 256].
    wq4t_f = np.concatenate([Wq.T] * 4, axis=1).astype(np.float16)   # [256, 128]
    wk4t_f = np.concatenate([Wk.T] * 4, axis=1).astype(np.float16)
    wvt_f = Wv.T.astype(np.float16)                                  # [256, 256]
    wall = np.ascontiguousarray(
        np.concatenate(
            [
                np.concatenate([wq4t_f[ch::2], wk4t_f[ch::2], wvt_f[ch::2]], axis=1)
                for ch in range(2)
            ],
            axis=1,
        )
    )  # [128, 1024]
    biases = np.ascontiguousarray(
        np.stack(
            [np.tile(bq, 4), np.tile(bk, 4), bv[:128], bv[128:]], axis=1
        ).astype(np.float32)
    )  # [128, 4]
    import ml_dtypes

    ones_c = np.ones((128, 32), ml_dtypes.bfloat16)
    ones_f = np.ones((128, 128), np.float32)

    in_maps = []
    for core in range(8):
        b, h = divmod(core, 2)
        in_maps.append(
            dict(
                # [C, cols] -> [128, 2*cols]: rows (2p, 2p+1) concatenated per
                # partition; a plain reshape since channel rows are adjacent.
                x1=np.ascontiguousarray(x1[b, :, h * NQ : (h + 1) * NQ]).reshape(
                    128, 2 * NQ
                ),
                x2=np.ascontiguousarray(x2[b]).reshape(128, 2 * N),
                wall=wall,
                biases=biases,
                ones_c=ones_c,
                ones_f=ones_f,
            )
        )
    return in_maps


_NC_CACHE = None


def _get_nc():
    global _NC_CACHE
    if _NC_CACHE is None:
        _NC_CACHE = build_nc()
    return _NC_CACHE


def kernel(f1, f2, Wq, bq, Wk, bk, Wv, bv):
    in_maps = make_in_maps(f1, f2, Wq, bq, Wk, bk, Wv, bv)
    res = bass_utils.run_bass_kernel_spmd(_get_nc(), in_maps, core_ids=list(range(8)))
    out = np.empty((B, C, N), np.float32)
    for core in range(8):
        b, h = divmod(core, 2)
        out[b, :, h * NQ : (h + 1) * NQ] = res.results[core]["out"].astype(np.float32)
    return out.reshape(B, C, 64, 64)



# revision 6
# speedup vs baseline: 1.1254x; 1.1254x over previous
"""Cross-attention Trainium2 kernel (nn_CrossAttention).

Reference computation (per batch b):
    q = Wq @ x1 + bq            [32, N]     (N = 64*64 = 4096)
    k = Wk @ x2 + bk            [32, N]
    v = Wv @ x2 + bv            [256, N]
    attn = softmax(q^T k, axis over keys m)     [N, N]
    out[c, n] = sum_m v[c, m] attn[n, m]        [256, N]

Sharding: 8 cores = 4 batches x 2 query-halves (2048 queries per core, all
4096 keys).  Each core runs the same NEFF on its own input slice; softmax
rows are complete within a core so no cross-core communication is needed.

Per-core kernel layout choices (v2):
  * x1/x2/weights fp16 on host: projections at full PE rate, half the HBM
    traffic, fp16's 11-bit mantissa keeps logits accurate.
  * q/k stored fp16 (not f32r): the QK^T matmuls run at the 1-row/cycle
    16-bit PE rate (~216ns per 512-col pair) instead of fp32_mode=HIGH
    (~490ns).  fp16 q/k rounding adds ~4e-3 absolute logit error - noise
    next to the fp16 projection-input error already present.
  * S^T tiles [keys m on partitions, queries n free] so exp(S^T) feeds the
    AV matmuls directly as the stationary operand.
  * AV computed TRANSPOSED: psum[q quarter 128, 257] = pt[keys, q]^T @
    vt_aug[keys, 257], where vt_aug column 256 is constant 1.0 so the
    softmax row-sum accumulates as a free 257th channel.  This removes the
    col-packed rowsum matmuls, the partial-gather DMA and the broadcast
    matmul of v1 (~14us of PE time).
  * Normalization is then one fused DVE op per query quarter:
    out_f16 = (psum[:, 0:256] * recip(psum[:, 256])) + bv_bcast.
  * exp stays on ACT alone (bf16 out, exact); ACT does nothing else in the
    main loop.  ~2.22us/step of exp paces against ~2.2us/step of PE work.
  * Startup: all five engines issue DMAs in parallel right after the
    preamble (sync/vector: x1 halves; scalar: weights+consts; tensor/
    gpsimd: x2 key-blocks) so Q-proj starts ~10us and prep streams per
    1024-key block instead of serializing on two queues (~40us -> ~17us).
  * Output is written fp16 TRANSPOSED [2048 q, 256 c]; the host transposes,
    upcasts and adds bv (softmax rows sum to 1 so bv folds in post-norm).
  * Softmax skips the max-subtraction: logits ~N(0, 32), exp stays inside
    fp32/bf16 range.
"""

import sys

for _p in (
    "/root/.axon_site",
    "/root/.axon_site/_ro/trn_rl_repo",
    "/root/.axon_site/_ro/pypackages",
):
    if _p not in sys.path:
        sys.path.append(_p)

import numpy as np

import concourse.bass as bass
from concourse import bacc
import concourse.tile as tile
from concourse import mybir
from concourse import bass_utils

B = 4
C = 256          # value/input channels
D = 32           # q/k channels
N = 4096         # keys per batch (64*64)
NQ = 2048        # queries per core (half a batch)
NT = 512         # query tile (free dim of S^T)
NNT = NQ // NT   # 4 query tiles
NSC = 8          # key super-chunks of 512 (4 x 128) keys
VSTRIDE = 258    # vt chunk stride: 256 channels + ones col + pad
F32 = mybir.dt.float32
F16 = mybir.dt.float16
BF16 = mybir.dt.bfloat16
AFT = mybir.ActivationFunctionType
ALU = mybir.AluOpType


def attn_tile_kernel(tc, out, x1, x2, wall, biases, bvb):
    nc = tc.nc

    with (
        tc.tile_pool(name="consts", bufs=1) as consts,
        tc.tile_pool(name="bigbuf", bufs=1) as bigbuf,
        # 4 pt bufs: two halves are allocated per step BEFORE the previous
        # step's AV consumers are emitted; with <4 bufs a new exp would
        # reuse a buffer whose reader isn't emitted yet (untracked race).
        tc.tile_pool(name="ptbuf", bufs=4) as ptbuf,
        tc.tile_pool(name="finbuf", bufs=4) as finbuf,
    ):
        # ---- constants / weights -------------------------------------
        # biases packed as one [128, 2] f32: cols = bq4 | bk4
        bias_sb = consts.tile([128, 2], F32, name="bias_sb")
        nc.scalar.dma_start(out=bias_sb, in_=biases)
        # bv broadcast to all partitions [128, 256] f32 (for the fused
        # per-quarter normalize: out = psum*recip + bv)
        bvb_sb = consts.tile([128, C], F32, name="bvb_sb")
        nc.scalar.dma_start(out=bvb_sb, in_=bvb)
        bq4_sb = bias_sb[:, 0:1]
        bk4_sb = bias_sb[:, 1:2]

        # All weights ride in one contiguous [128, 1024] fp16 tensor,
        # host-interleaved to the channel-pair layout: plane ch holds
        # channels {2p+ch}, cols = [wq 128 | wk 128 | wv 256] per plane.
        # First on the scalar queue: everything in prep needs it.
        wall_sb = consts.tile([128, 1024], F16, name="wall_sb")
        nc.scalar.dma_start(out=wall_sb, in_=wall)
        wq4t_sb = [wall_sb[:, ch * 512 : ch * 512 + 128] for ch in range(2)]
        wk4t_sb = [wall_sb[:, ch * 512 + 128 : ch * 512 + 256] for ch in range(2)]
        wvt_sb = [wall_sb[:, ch * 512 + 256 : ch * 512 + 512] for ch in range(2)]

        # ---- feature maps -------------------------------------------
        # Host passes x1/x2 reshaped [128, 2*cols]: partition p holds the
        # channel pair (2p, 2p+1) back-to-back, so each DMA line is one
        # fully contiguous 8-16KB read.  Five engines issue in parallel:
        # sync + vector take the x1 partition halves (Q-proj is the
        # critical path), tensor + gpsimd stream x2 per 1024-key block and
        # channel plane (tensor idles until x1 lands anyway; gpsimd is
        # otherwise unused).
        x1_sb = bigbuf.tile([128, 2 * NQ], F16, name="x1_sb")
        x2_sb = bigbuf.tile([128, 2 * N], F16, name="x2_sb")
        nc.sync.dma_start(out=x1_sb[0:64, :], in_=x1[0:64, :])
        nc.scalar.dma_start(out=x1_sb[64:128, :], in_=x1[64:128, :])

        def x2blk(ap, ph, blk):
            # both channel planes of one 1024-key block, one partition half:
            # 3D AP [64, 2, 1024] (plane stride N, packed cols)
            return ap[ph * 64 : (ph + 1) * 64, :].rearrange(
                "p (c n) -> p c n", c=2
            )[:, :, blk * 1024 : (blk + 1) * 1024]

        for blk in range(4):
            nc.sync.dma_start(out=x2blk(x2_sb, 0, blk), in_=x2blk(x2, 0, blk))
        for blk in range(2):
            nc.scalar.dma_start(out=x2blk(x2_sb, 1, blk), in_=x2blk(x2, 1, blk))
        for blk in range(2, 4):
            nc.gpsimd.dma_start(out=x2blk(x2_sb, 1, blk), in_=x2blk(x2, 1, blk))

        def x1p(ch, cols):
            return x1_sb[:, ch * NQ + cols.start : ch * NQ + cols.stop]

        def x2p(ch, cols):
            return x2_sb[:, ch * N + cols.start : ch * N + cols.stop]

        q4_sb = bigbuf.tile([128, NQ], F16, name="q4_sb")
        k4_sb = bigbuf.tile([128, N], F16, name="k4_sb")
        # vt_aug: 32 chunks of [128 keys, 258]: 256 channels + 1.0 + pad
        vt_sb = bigbuf.tile([128, 32 * VSTRIDE], BF16, name="vt_sb")

        # ones columns of vt (col 256 of each chunk), set once on gpsimd
        # (idle during prep); disjoint from the psum_v evacuation columns.
        for mc in range(32):
            nc.gpsimd.memset(vt_sb[:, mc * VSTRIDE + 256 : mc * VSTRIDE + 257], 1.0)

        # ---- prep: projections ---------------------------------------
        # Interleaved per 1024-column x2 block so PE work becomes available
        # as each DMA block lands: Q4 first (x1), then per block K4 + V^T.
        with tc.tile_pool(name="prep_psum", bufs=4, space="PSUM") as pp:
            # Q4 [128, 2048] = (Wq stacked 4x) @ x1, then +bq (two psum tiles)
            for qh in range(2):
                psum_q = pp.tile([128, 1024], F32, name=f"psum_q{qh}", tag="prep")
                for t2 in range(2):
                    cols = slice(t2 * NT, (t2 + 1) * NT)
                    src_c = slice(qh * 1024 + t2 * NT, qh * 1024 + (t2 + 1) * NT)
                    for ch in range(2):
                        nc.tensor.matmul(
                            psum_q[:, cols],
                            lhsT=wq4t_sb[ch],
                            rhs=x1p(ch, src_c),
                            start=(ch == 0),
                            stop=(ch == 1),
                        )
                nc.vector.tensor_scalar_add(
                    q4_sb[:, qh * 1024 : (qh + 1) * 1024], psum_q, bq4_sb
                )

            for blk in range(4):
                bcols = slice(blk * 1024, (blk + 1) * 1024)
                # K4 for this block
                psum_k = pp.tile([128, 1024], F32, name=f"psum_k{blk}", tag="prep")
                for t2 in range(2):
                    cols = slice(t2 * NT, (t2 + 1) * NT)
                    src_c = slice(blk * 1024 + t2 * NT, blk * 1024 + (t2 + 1) * NT)
                    for ch in range(2):
                        nc.tensor.matmul(
                            psum_k[:, cols],
                            lhsT=wk4t_sb[ch],
                            rhs=x2p(ch, src_c),
                            start=(ch == 0),
                            stop=(ch == 1),
                        )
                nc.vector.tensor_scalar_add(k4_sb[:, bcols], psum_k, bk4_sb)
                # V^T for this block's 8 m-chunks, two psum tiles of 4
                for half in range(2):
                    psum_v = pp.tile(
                        [128, 1024], F32, name=f"psum_v{blk}_{half}", tag="prep"
                    )
                    for m4 in range(4):
                        mc = 8 * blk + 4 * half + m4
                        for ch in range(2):
                            nc.tensor.matmul(
                                psum_v[:, m4 * 256 : (m4 + 1) * 256],
                                lhsT=x2p(ch, slice(mc * 128, (mc + 1) * 128)),
                                rhs=wvt_sb[ch],
                                start=(ch == 0),
                                stop=(ch == 1),
                            )
                    # evacuate to the 258-strided vt layout in one strided
                    # copy; scalar (ACT is idle in prep) and vector split
                    # the halves.
                    mc0 = 8 * blk + 4 * half
                    dst = vt_sb[:, mc0 * VSTRIDE : (mc0 + 4) * VSTRIDE]
                    dst4 = dst.rearrange("p (c w) -> p c w", c=4)[:, :, 0:256]
                    src4 = psum_v.rearrange("p (c w) -> p c w", c=4)
                    if half == 0:
                        nc.scalar.copy(dst4, src4)
                    else:
                        nc.vector.tensor_copy(dst4, src4)

        # ---- main attention loop -------------------------------------
        # Flat software pipeline over (nt, sc) steps, each split into two
        # halves h of 2 key-chunks.  The S^T psum is a 2-deep pool of
        # [128, 1024] halves, so exp (ACT) of half (i, h) overlaps the S^T
        # matmuls of the next half/step on the PE instead of serializing.
        # AV matmuls of step i are emitted after step i+1's S^T, so the PE
        # always has work while ACT computes exp.
        with (
            tc.tile_pool(name="s_psum", bufs=2, space="PSUM") as sp,
            tc.tile_pool(name="o_psum", bufs=1, space="PSUM") as op,
        ):
            state = {}

            def _emit_st_half(nt, sc, h):
                # S^T half: 2 row-packed matmuls (chunks 4*sc+2h+{0,1}) at
                # row-groups {2h, 2h+1}; exp -> bf16 pt on ACT.
                qcols = slice(nt * NT, (nt + 1) * NT)
                psum_s = sp.tile([128, 2 * NT], F32, name=f"ps_{nt}_{sc}_{h}", tag="s")
                for j in range(2):
                    mc = 4 * sc + 2 * h + j
                    rowg = slice(32 * (2 * h + j), 32 * (2 * h + j + 1))
                    nc.tensor.matmul(
                        psum_s[:, j * NT : (j + 1) * NT],
                        lhsT=k4_sb[rowg, mc * 128 : (mc + 1) * 128],
                        rhs=q4_sb[rowg, qcols],
                        start=True,
                        stop=True,
                        tile_position=(32 * (2 * h + j), 0),
                    )
                pt = ptbuf.tile([128, 2 * NT], BF16, name=f"pt_{nt}_{sc}_{h}", tag="pt")
                nc.scalar.activation(out=pt, in_=psum_s, func=AFT.Exp)
                return pt

            def _emit_av(nt, sc, pt_a, pt_b, last_tile=False):
                first, last = sc == 0, sc == NSC - 1
                if first:
                    state[nt] = [
                        op.tile([128, C + 1], F32, name=f"po{qq}_{nt}", tag=f"o{qq}")
                        for qq in range(4)
                    ]
                psums = state[nt]
                # chunk-outer keeps all 4 quarters' accumulation spread; for
                # the last tile go quarter-outer so fin(qq) unblocks after
                # its own 4 chunk matmuls instead of all 16.
                if last_tile and last:
                    order = [(cx, qq) for qq in range(4) for cx in range(4)]
                else:
                    order = [(cx, qq) for cx in range(4) for qq in range(4)]
                for cx, qq in order:
                    h, j = divmod(cx, 2)
                    pt = (pt_a, pt_b)[h]
                    mc = 4 * sc + 2 * h + j
                    nc.tensor.matmul(
                        psums[qq],
                        lhsT=pt[:, j * NT + qq * 128 : j * NT + (qq + 1) * 128],
                        rhs=vt_sb[:, mc * VSTRIDE : mc * VSTRIDE + C + 1],
                        start=(first and cx == 0),
                        stop=(last and cx == 3),
                    )

            def _emit_fin(nt):
                # per query quarter: recip of the ones-channel rowsum, then
                # one fused DVE op: out_f16 = psum[:,0:256]*recip + bv.
                psums = state.pop(nt)
                for qq in range(4):
                    po = psums[qq]
                    rcp = finbuf.tile([128, 1], F32, name=f"rcp_{nt}_{qq}", tag="rcp")
                    nc.vector.reciprocal_approx_fast(out=rcp, in_=po[:, C : C + 1])
                    o16 = finbuf.tile([128, C], F16, name=f"o16_{nt}_{qq}", tag="o16")
                    nc.vector.scalar_tensor_tensor(
                        out=o16,
                        in0=po[:, 0:C],
                        scalar=rcp,
                        in1=bvb_sb,
                        op0=ALU.mult,
                        op1=ALU.add,
                    )
                    rowq = nt * NT + qq * 128
                    (nc.sync, nc.gpsimd)[qq % 2].dma_start(
                        out=out[rowq : rowq + 128, :], in_=o16
                    )

            steps = [(nt, sc) for nt in range(NNT) for sc in range(NSC)]
            prev = None
            for nt, sc in steps:
                pt_a = _emit_st_half(nt, sc, 0)
                pt_b = _emit_st_half(nt, sc, 1)
                if prev is not None:
                    pnt, psc, ppa, ppb = prev
                    _emit_av(pnt, psc, ppa, ppb)
                    if psc == NSC - 1:
                        _emit_fin(pnt)
                prev = (nt, sc, pt_a, pt_b)
            pnt, psc, ppa, ppb = prev
            _emit_av(pnt, psc, ppa, ppb, last_tile=True)
            _emit_fin(pnt)


def build_nc():
    nc = bacc.Bacc("TRN2", target_bir_lowering=False, debug=False)
    x1 = nc.dram_tensor("x1", [128, 2 * NQ], F16, kind="ExternalInput").ap()
    x2 = nc.dram_tensor("x2", [128, 2 * N], F16, kind="ExternalInput").ap()
    wall = nc.dram_tensor("wall", [128, 1024], F16, kind="ExternalInput").ap()
    biases = nc.dram_tensor("biases", [128, 2], F32, kind="ExternalInput").ap()
    bvb = nc.dram_tensor("bvb", [128, C], F32, kind="ExternalInput").ap()
    out = nc.dram_tensor("out", [NQ, C], F16, kind="ExternalOutput").ap()
    with tile.TileContext(nc) as tc:
        attn_tile_kernel(tc, out, x1, x2, wall, biases, bvb)
    nc.compile()
    return nc


def make_in_maps(f1, f2, Wq, bq, Wk, bk, Wv, bv):
    f1 = np.asarray(f1, dtype=np.float32)
    f2 = np.asarray(f2, dtype=np.float32)
    Wq = np.asarray(Wq, dtype=np.float32)
    Wk = np.asarray(Wk, dtype=np.float32)
    Wv = np.asarray(Wv, dtype=np.float32)
    bq = np.asarray(bq, dtype=np.float32)
    bk = np.asarray(bk, dtype=np.float32)
    bv = np.asarray(bv, dtype=np.float32)

    x1 = f1.reshape(B, C, N).astype(np.float16)
    x2 = f2.reshape(B, C, N).astype(np.float16)
    # channel-pair packing: SBUF partition p holds channels (2p, 2p+1), so
    # weight plane ch = rows (2p+ch) of W^T.  All weights merge into one
    # [128, 1024] fp16 tensor: per plane [wq4 128 | wk4 128 | wv 256].
    wq4t_f = np.concatenate([Wq.T] * 4, axis=1).astype(np.float16)   # [256, 128]
    wk4t_f = np.concatenate([Wk.T] * 4, axis=1).astype(np.float16)
    wvt_f = Wv.T.astype(np.float16)                                  # [256, 256]
    wall = np.ascontiguousarray(
        np.concatenate(
            [
                np.concatenate([wq4t_f[ch::2], wk4t_f[ch::2], wvt_f[ch::2]], axis=1)
                for ch in range(2)
            ],
            axis=1,
        )
    )  # [128, 1024]
    biases = np.ascontiguousarray(
        np.stack([np.tile(bq, 4), np.tile(bk, 4)], axis=1).astype(np.float32)
    )  # [128, 2]
    bvb = np.ascontiguousarray(np.broadcast_to(bv[None, :], (128, C))).astype(
        np.float32
    )

    in_maps = []
    for core in range(8):
        b, h = divmod(core, 2)
        in_maps.append(
            dict(
                # [C, cols] -> [128, 2*cols]: rows (2p, 2p+1) concatenated per
                # partition; a plain reshape since channel rows are adjacent.
                x1=np.ascontiguousarray(x1[b, :, h * NQ : (h + 1) * NQ]).reshape(
                    128, 2 * NQ
                ),
                x2=np.ascontiguousarray(x2[b]).reshape(128, 2 * N),
                wall=wall,
                biases=biases,
                bvb=bvb,
            )
        )
    return in_maps


_NC_CACHE = None


def _get_nc():
    global _NC_CACHE
    if _NC_CACHE is None:
        _NC_CACHE = build_nc()
    return _NC_CACHE


def kernel(f1, f2, Wq, bq, Wk, bk, Wv, bv):
    in_maps = make_in_maps(f1, f2, Wq, bq, Wk, bk, Wv, bv)
    res = bass_utils.run_bass_kernel_spmd(_get_nc(), in_maps, core_ids=list(range(8)))
    bv32 = np.asarray(bv, dtype=np.float32)[:, None]
    out = np.empty((B, C, N), np.float32)
    for core in range(8):
        b, h = divmod(core, 2)
        # device emits transposed [2048 q, 256 c] fp16 without bv
        oT = res.results[core]["out"].astype(np.float32)
        out[b, :, h * NQ : (h + 1) * NQ] = oT.T + bv32
    return out.reshape(B, C, 64, 64)


# revision 14
# speedup vs baseline: 1.1648x; 1.0350x over previous
"""Cross-attention Trainium2 kernel (nn_CrossAttention).

Reference computation (per batch b):
    q = Wq @ x1 + bq            [32, N]     (N = 64*64 = 4096)
    k = Wk @ x2 + bk            [32, N]
    v = Wv @ x2 + bv            [256, N]
    attn = softmax(q^T k, axis over keys m)     [N, N]
    out[c, n] = sum_m v[c, m] attn[n, m]        [256, N]

Sharding: 8 cores = 4 batches x 2 query-halves (2048 queries per core, all
4096 keys).  Each core runs the same NEFF on its own input slice; softmax
rows are complete within a core so no cross-core communication is needed.

Per-core kernel layout choices (v2):
  * x1/x2/weights fp16 on host: projections at full PE rate, half the HBM
    traffic, fp16's 11-bit mantissa keeps logits accurate.
  * q/k stored fp16 (not f32r): the QK^T matmuls run at the 1-row/cycle
    16-bit PE rate (~216ns per 512-col pair) instead of fp32_mode=HIGH
    (~490ns).  fp16 q/k rounding adds ~4e-3 absolute logit error - noise
    next to the fp16 projection-input error already present.
  * S^T tiles [keys m on partitions, queries n free] so exp(S^T) feeds the
    AV matmuls directly as the stationary operand.
  * AV computed TRANSPOSED: psum[q quarter 128, 257] = pt[keys, q]^T @
    vt_aug[keys, 257], where vt_aug column 256 is constant 1.0 so the
    softmax row-sum accumulates as a free 257th channel.  This removes the
    col-packed rowsum matmuls, the partial-gather DMA and the broadcast
    matmul of v1 (~14us of PE time).
  * Normalization is then one fused DVE op per query quarter:
    out_f16 = (psum[:, 0:256] * recip(psum[:, 256])) + bv_bcast.
  * exp stays on ACT alone (bf16 out, exact); ACT does nothing else in the
    main loop.  ~2.22us/step of exp paces against ~2.2us/step of PE work.
  * Startup: all five engines issue DMAs in parallel right after the
    preamble (sync/vector: x1 halves; scalar: weights+consts; tensor/
    gpsimd: x2 key-blocks) so Q-proj starts ~10us and prep streams per
    1024-key block instead of serializing on two queues (~40us -> ~17us).
  * Output is written fp16 TRANSPOSED [2048 q, 256 c]; the host transposes,
    upcasts and adds bv (softmax rows sum to 1 so bv folds in post-norm).
  * Softmax skips the max-subtraction: logits ~N(0, 32), exp stays inside
    fp32/bf16 range.
"""

import sys

for _p in (
    "/root/.axon_site",
    "/root/.axon_site/_ro/trn_rl_repo",
    "/root/.axon_site/_ro/pypackages",
):
    if _p not in sys.path:
        sys.path.append(_p)

import numpy as np

import concourse.bass as bass
from concourse import bacc
import concourse.tile as tile
from concourse import mybir
from concourse import bass_utils

B = 4
C = 256          # value/input channels
D = 32           # q/k channels
N = 4096         # keys per batch (64*64)
NQ = 2048        # queries per core (half a batch)
NT = 512         # query tile (free dim of S^T)
NNT = NQ // NT   # 4 query tiles
NSC = 8          # key super-chunks of 512 (4 x 128) keys
VSTRIDE = 258    # vt chunk stride: 256 channels + ones col + pad
F32 = mybir.dt.float32
F16 = mybir.dt.float16
BF16 = mybir.dt.bfloat16
AFT = mybir.ActivationFunctionType
ALU = mybir.AluOpType


def attn_tile_kernel(tc, out, x1, x2, wall, biases):
    nc = tc.nc

    with (
        tc.tile_pool(name="consts", bufs=1) as consts,
        tc.tile_pool(name="bigbuf", bufs=1) as bigbuf,
        # 4 pt bufs: two halves are allocated per step BEFORE the previous
        # step's AV consumers are emitted; with <4 bufs a new exp would
        # reuse a buffer whose reader isn't emitted yet (untracked race).
        tc.tile_pool(name="ptbuf", bufs=4) as ptbuf,
        tc.tile_pool(name="finbuf", bufs=4) as finbuf,
    ):
        # ---- constants / weights -------------------------------------
        # All weights ride in one contiguous [128, 1024] fp16 tensor,
        # host-interleaved to the channel-pair layout: plane ch holds
        # channels {2p+ch}, cols = [wq 128 | wk 128 | wv 256] per plane.
        # First on the scalar queue: everything in prep needs it.
        wall_sb = consts.tile([128, 1024], F16, name="wall_sb")
        nc.scalar.dma_start(out=wall_sb, in_=wall)
        # x1 second half rides the scalar queue RIGHT after wall (Q-proj is
        # the critical path; tiny-line const DMAs go after).
        x1_sb = bigbuf.tile([128, 2 * NQ], F16, name="x1_sb")
        nc.sync.dma_start(out=x1_sb[0:64, :], in_=x1[0:64, :])
        nc.scalar.dma_start(out=x1_sb[64:128, :], in_=x1[64:128, :])

        # biases + bv broadcast in ONE [128, 258] f32 tensor (1032B lines;
        # separate tiny DMAs with 8B lines stall the queue for microseconds):
        # col 0 = bq4, col 1 = bk4, cols 2:258 = bv broadcast to all rows.
        cst_sb = consts.tile([128, 2 + C], F32, name="cst_sb")
        nc.scalar.dma_start(out=cst_sb, in_=biases)
        bq4_sb = cst_sb[:, 0:1]
        bk4_sb = cst_sb[:, 1:2]
        bvb_sb = cst_sb[:, 2 : 2 + C]
        wq4t_sb = [wall_sb[:, ch * 512 : ch * 512 + 128] for ch in range(2)]
        wk4t_sb = [wall_sb[:, ch * 512 + 128 : ch * 512 + 256] for ch in range(2)]
        wvt_sb = [wall_sb[:, ch * 512 + 256 : ch * 512 + 512] for ch in range(2)]

        # ---- feature maps -------------------------------------------
        # Host passes x1/x2 reshaped [128, 2*cols]: partition p holds the
        # channel pair (2p, 2p+1) back-to-back, so each DMA line is one
        # fully contiguous 8-16KB read.  Five engines issue in parallel:
        # sync + vector take the x1 partition halves (Q-proj is the
        # critical path), tensor + gpsimd stream x2 per 1024-key block and
        # channel plane (tensor idles until x1 lands anyway; gpsimd is
        # otherwise unused).
        x2_sb = bigbuf.tile([128, 2 * N], F16, name="x2_sb")

        def x2blk(ap, ph, blk):
            # both channel planes of one 1024-key block, one partition half:
            # 3D AP [64, 2, 1024] (plane stride N, packed cols)
            return ap[ph * 64 : (ph + 1) * 64, :].rearrange(
                "p (c n) -> p c n", c=2
            )[:, :, blk * 1024 : (blk + 1) * 1024]

        for blk in range(4):
            nc.sync.dma_start(out=x2blk(x2_sb, 0, blk), in_=x2blk(x2, 0, blk))
        for blk in range(2):
            nc.scalar.dma_start(out=x2blk(x2_sb, 1, blk), in_=x2blk(x2, 1, blk))
        for blk in range(2, 4):
            nc.gpsimd.dma_start(out=x2blk(x2_sb, 1, blk), in_=x2blk(x2, 1, blk))

        def x1p(ch, cols):
            return x1_sb[:, ch * NQ + cols.start : ch * NQ + cols.stop]

        def x2p(ch, cols):
            return x2_sb[:, ch * N + cols.start : ch * N + cols.stop]

        q4_sb = bigbuf.tile([128, NQ], F16, name="q4_sb")
        k4_sb = bigbuf.tile([128, N], F16, name="k4_sb")
        # vt_aug: 32 chunks of [128 keys, 258]: 256 channels + 1.0 + pad
        vt_sb = bigbuf.tile([128, 32 * VSTRIDE], BF16, name="vt_sb")

        # ones columns of vt (col 256 of each chunk), set once on gpsimd
        # (idle during prep); disjoint from the psum_v evacuation columns.
        for mc in range(32):
            nc.gpsimd.memset(vt_sb[:, mc * VSTRIDE + 256 : mc * VSTRIDE + 257], 1.0)

        # ---- prep: projections ---------------------------------------
        # Interleaved per 1024-column x2 block so PE work becomes available
        # as each DMA block lands: Q4 first (x1), then per block K4 + V^T.
        with tc.tile_pool(name="prep_psum", bufs=4, space="PSUM") as pp:
            # PE p-state warmup: the tensor clock ramps with sustained use
            # (~2x slower for the first ~3us).  Burn the otherwise-idle DMA
            # wait on dummy matmuls over a zeroed scratch tile so the real
            # projections start at full speed.
            wsc = consts.tile([128, 512], F16, name="warm_sc")
            nc.vector.memset(wsc, 0.0)
            psum_w = pp.tile([128, 512], F32, name="psum_w", tag="prep")
            for _ in range(10):
                nc.tensor.matmul(
                    psum_w, lhsT=wsc[:, 0:128], rhs=wsc, start=True, stop=True
                )
            # Q4 [128, 2048] = (Wq stacked 4x) @ x1, then +bq (two psum tiles)
            for qh in range(2):
                psum_q = pp.tile([128, 1024], F32, name=f"psum_q{qh}", tag="prep")
                for t2 in range(2):
                    cols = slice(t2 * NT, (t2 + 1) * NT)
                    src_c = slice(qh * 1024 + t2 * NT, qh * 1024 + (t2 + 1) * NT)
                    for ch in range(2):
                        nc.tensor.matmul(
                            psum_q[:, cols],
                            lhsT=wq4t_sb[ch],
                            rhs=x1p(ch, src_c),
                            start=(ch == 0),
                            stop=(ch == 1),
                        )
                nc.vector.tensor_scalar_add(
                    q4_sb[:, qh * 1024 : (qh + 1) * 1024], psum_q, bq4_sb
                )

            for blk in range(4):
                bcols = slice(blk * 1024, (blk + 1) * 1024)
                # K4 for this block
                psum_k = pp.tile([128, 1024], F32, name=f"psum_k{blk}", tag="prep")
                for t2 in range(2):
                    cols = slice(t2 * NT, (t2 + 1) * NT)
                    src_c = slice(blk * 1024 + t2 * NT, blk * 1024 + (t2 + 1) * NT)
                    for ch in range(2):
                        nc.tensor.matmul(
                            psum_k[:, cols],
                            lhsT=wk4t_sb[ch],
                            rhs=x2p(ch, src_c),
                            start=(ch == 0),
                            stop=(ch == 1),
                        )
                nc.vector.tensor_scalar_add(k4_sb[:, bcols], psum_k, bk4_sb)
                # V^T for this block's 8 m-chunks, two psum tiles of 4
                for half in range(2):
                    psum_v = pp.tile(
                        [128, 1024], F32, name=f"psum_v{blk}_{half}", tag="prep"
                    )
                    for m4 in range(4):
                        mc = 8 * blk + 4 * half + m4
                        for ch in range(2):
                            nc.tensor.matmul(
                                psum_v[:, m4 * 256 : (m4 + 1) * 256],
                                lhsT=x2p(ch, slice(mc * 128, (mc + 1) * 128)),
                                rhs=wvt_sb[ch],
                                start=(ch == 0),
                                stop=(ch == 1),
                            )
                    # evacuate to the 258-strided vt layout in one strided
                    # copy; scalar (ACT is idle in prep) and vector split
                    # the halves.
                    mc0 = 8 * blk + 4 * half
                    dst = vt_sb[:, mc0 * VSTRIDE : (mc0 + 4) * VSTRIDE]
                    dst4 = dst.rearrange("p (c w) -> p c w", c=4)[:, :, 0:256]
                    src4 = psum_v.rearrange("p (c w) -> p c w", c=4)
                    if half == 0:
                        nc.scalar.copy(dst4, src4)
                    else:
                        nc.vector.tensor_copy(dst4, src4)

        # ---- main attention loop -------------------------------------
        # Flat software pipeline over (nt, sc) steps, each split into two
        # halves h of 2 key-chunks.  The S^T psum is a 2-deep pool of
        # [128, 1024] halves, so exp (ACT) of half (i, h) overlaps the S^T
        # matmuls of the next half/step on the PE instead of serializing.
        # AV matmuls of step i are emitted after step i+1's S^T, so the PE
        # always has work while ACT computes exp.
        with (
            tc.tile_pool(name="s_psum", bufs=2, space="PSUM") as sp,
            tc.tile_pool(name="o_psum", bufs=1, space="PSUM") as op,
        ):
            state = {}

            def _emit_st(nt, sc):
                # S^T step: 4 row-packed matmuls (key chunks 4*sc+g at PE
                # row-groups g*32) emitted back-to-back so all four
                # co-execute in one rhs streaming pass; 2 psum halves, then
                # exp of each half -> bf16 pt on ACT.
                qcols = slice(nt * NT, (nt + 1) * NT)
                ps = [
                    sp.tile([128, 2 * NT], F32, name=f"ps_{nt}_{sc}_{h}", tag="s")
                    for h in range(2)
                ]
                for g in range(4):
                    h, j = divmod(g, 2)
                    mc = 4 * sc + g
                    rowg = slice(32 * g, 32 * (g + 1))
                    nc.tensor.matmul(
                        ps[h][:, j * NT : (j + 1) * NT],
                        lhsT=k4_sb[rowg, mc * 128 : (mc + 1) * 128],
                        rhs=q4_sb[rowg, qcols],
                        start=True,
                        stop=True,
                        tile_position=(32 * g, 0),
                    )
                pts = []
                for h in range(2):
                    pt = ptbuf.tile(
                        [128, 2 * NT], BF16, name=f"pt_{nt}_{sc}_{h}", tag="pt"
                    )
                    nc.scalar.activation(out=pt, in_=ps[h], func=AFT.Exp)
                    pts.append(pt)
                return pts

            def _fin_quarter(nt, qq, po):
                # recip of the ones-channel rowsum, then one fused DVE op:
                # out_f16 = psum[:,0:256]*recip + bv.
                rcp = finbuf.tile([128, 1], F32, name=f"rcp_{nt}_{qq}", tag="rcp")
                nc.vector.reciprocal_approx_fast(out=rcp, in_=po[:, C : C + 1])
                o16 = finbuf.tile([128, C], F16, name=f"o16_{nt}_{qq}", tag="o16")
                nc.vector.scalar_tensor_tensor(
                    out=o16,
                    in0=po[:, 0:C],
                    scalar=rcp,
                    in1=bvb_sb,
                    op0=ALU.mult,
                    op1=ALU.add,
                )
                rowq = nt * NT + qq * 128
                (nc.sync, nc.gpsimd)[qq % 2].dma_start(
                    out=out[rowq : rowq + 128, :], in_=o16
                )

            def _emit_av(nt, sc, pt_a, pt_b, last_tile=False):
                first, last = sc == 0, sc == NSC - 1
                if first:
                    state[nt] = [
                        op.tile([128, C + 1], F32, name=f"po{qq}_{nt}", tag=f"o{qq}")
                        for qq in range(4)
                    ]
                psums = state[nt]
                # chunk-outer keeps all 4 quarters' accumulation spread; for
                # the last tile go quarter-outer with fin(qq) right behind
                # each quarter's last matmul so the tail pipelines.
                if last_tile and last:
                    for qq in range(4):
                        for cx in range(4):
                            h, j = divmod(cx, 2)
                            mc = 4 * sc + 2 * h + j
                            nc.tensor.matmul(
                                psums[qq],
                                lhsT=(pt_a, pt_b)[h][
                                    :, j * NT + qq * 128 : j * NT + (qq + 1) * 128
                                ],
                                rhs=vt_sb[:, mc * VSTRIDE : mc * VSTRIDE + C + 1],
                                start=False,
                                stop=(cx == 3),
                            )
                        _fin_quarter(nt, qq, psums[qq])
                    state.pop(nt)
                    return
                for cx in range(4):
                    h, j = divmod(cx, 2)
                    pt = (pt_a, pt_b)[h]
                    mc = 4 * sc + 2 * h + j
                    for qq in range(4):
                        nc.tensor.matmul(
                            psums[qq],
                            lhsT=pt[:, j * NT + qq * 128 : j * NT + (qq + 1) * 128],
                            rhs=vt_sb[:, mc * VSTRIDE : mc * VSTRIDE + C + 1],
                            start=(first and cx == 0),
                            stop=(last and cx == 3),
                        )

            def _emit_fin(nt):
                psums = state.pop(nt)
                for qq in range(4):
                    _fin_quarter(nt, qq, psums[qq])

            steps = [(nt, sc) for nt in range(NNT) for sc in range(NSC)]
            prev = None
            for nt, sc in steps:
                pt_a, pt_b = _emit_st(nt, sc)
                if prev is not None:
                    pnt, psc, ppa, ppb = prev
                    _emit_av(pnt, psc, ppa, ppb)
                    if psc == NSC - 1:
                        _emit_fin(pnt)
                prev = (nt, sc, pt_a, pt_b)
            pnt, psc, ppa, ppb = prev
            _emit_av(pnt, psc, ppa, ppb, last_tile=True)


def build_nc():
    nc = bacc.Bacc("TRN2", target_bir_lowering=False, debug=False)
    x1 = nc.dram_tensor("x1", [128, 2 * NQ], F16, kind="ExternalInput").ap()
    x2 = nc.dram_tensor("x2", [128, 2 * N], F16, kind="ExternalInput").ap()
    wall = nc.dram_tensor("wall", [128, 1024], F16, kind="ExternalInput").ap()
    biases = nc.dram_tensor("biases", [128, 2 + C], F32, kind="ExternalInput").ap()
    out = nc.dram_tensor("out", [NQ, C], F16, kind="ExternalOutput").ap()
    with tile.TileContext(nc) as tc:
        attn_tile_kernel(tc, out, x1, x2, wall, biases)
    nc.compile()
    return nc


def make_in_maps(f1, f2, Wq, bq, Wk, bk, Wv, bv):
    f1 = np.asarray(f1, dtype=np.float32)
    f2 = np.asarray(f2, dtype=np.float32)
    Wq = np.asarray(Wq, dtype=np.float32)
    Wk = np.asarray(Wk, dtype=np.float32)
    Wv = np.asarray(Wv, dtype=np.float32)
    bq = np.asarray(bq, dtype=np.float32)
    bk = np.asarray(bk, dtype=np.float32)
    bv = np.asarray(bv, dtype=np.float32)

    x1 = f1.reshape(B, C, N).astype(np.float16)
    x2 = f2.reshape(B, C, N).astype(np.float16)
    # channel-pair packing: SBUF partition p holds channels (2p, 2p+1), so
    # weight plane ch = rows (2p+ch) of W^T.  All weights merge into one
    # [128, 1024] fp16 tensor: per plane [wq4 128 | wk4 128 | wv 256].
    wq4t_f = np.concatenate([Wq.T] * 4, axis=1).astype(np.float16)   # [256, 128]
    wk4t_f = np.concatenate([Wk.T] * 4, axis=1).astype(np.float16)
    wvt_f = Wv.T.astype(np.float16)                                  # [256, 256]
    wall = np.ascontiguousarray(
        np.concatenate(
            [
                np.concatenate([wq4t_f[ch::2], wk4t_f[ch::2], wvt_f[ch::2]], axis=1)
                for ch in range(2)
            ],
            axis=1,
        )
    )  # [128, 1024]
    # col 0 = bq4, col 1 = bk4, cols 2:258 = bv broadcast (one DMA, 1KB lines)
    biases = np.ascontiguousarray(
        np.concatenate(
            [
                np.stack([np.tile(bq, 4), np.tile(bk, 4)], axis=1),
                np.broadcast_to(bv[None, :], (128, C)),
            ],
            axis=1,
        ).astype(np.float32)
    )  # [128, 258]

    in_maps = []
    for core in range(8):
        b, h = divmod(core, 2)
        in_maps.append(
            dict(
                # [C, cols] -> [128, 2*cols]: rows (2p, 2p+1) concatenated per
                # partition; a plain reshape since channel rows are adjacent.
                x1=np.ascontiguousarray(x1[b, :, h * NQ : (h + 1) * NQ]).reshape(
                    128, 2 * NQ
                ),
                x2=np.ascontiguousarray(x2[b]).reshape(128, 2 * N),
                wall=wall,
                biases=biases,
            )
        )
    return in_maps


_NC_CACHE = None


def _get_nc():
    global _NC_CACHE
    if _NC_CACHE is None:
        _NC_CACHE = build_nc()
    return _NC_CACHE


def kernel(f1, f2, Wq, bq, Wk, bk, Wv, bv):
    in_maps = make_in_maps(f1, f2, Wq, bq, Wk, bk, Wv, bv)
    res = bass_utils.run_bass_kernel_spmd(_get_nc(), in_maps, core_ids=list(range(8)))
    bv32 = np.asarray(bv, dtype=np.float32)[:, None]
    out = np.empty((B, C, N), np.float32)
    for core in range(8):
        b, h = divmod(core, 2)
        # device emits transposed [2048 q, 256 c] fp16 without bv
        oT = res.results[core]["out"].astype(np.float32)
        out[b, :, h * NQ : (h + 1) * NQ] = oT.T + bv32
    return out.reshape(B, C, 64, 64)
